# revision 24
# baseline (speedup 1.0000x reference)
"""Trainium2 Bass kernel for multi-head attention (B=8, N=1024, C=768, H=12, D=64).

Sharding: pure data parallelism — one batch element per NeuronCore (8 cores).
Each core computes qkv projection, softmax attention and output projection for
its [1024, 768] slice with full (replicated) weights. No collectives.

Dataflow (all "transposed" so no on-device transposes are needed):
  - host supplies xT = x[b].T (bf16) and w_qk pre-tiled [KT, 2H, 128, 128]
  - qk-pass:  qkT = w_qk.T @ x.T -> [1536, 1024]; head-pair t's tiles are
              computed during attention of pair t-1 (PE gap filling)
  - v-pass:   v = x @ w_v natural [1024, 768], computed inside pair 0/2 loops
  - ST pass:  ST[k,q] = (K Q^T) per head
  - exp:      PuT = exp(SCALE * ST) on ScalarE (no max subtraction: |S|<~7)
  - O pass:   O^T[d,q]: lhsT=[v | ones] so PSUM row 64 accumulates the softmax
              denominator l; two q-tile sweeps to keep PSUM pressure low
  - normalize: per head pair and q-tile, O^T *= (1/l)
  - proj:     yT = w_proj.T @ O^T + b; k-tiles 0..4 pre-accumulated and spilled
              so only the k=5 matmuls wait for the last attention pair
  - host transposes yT back.
"""

import sys

sys.path.insert(0, "/opt/trn_rl_repo")

import numpy as np

B, N, C = 8, 1024, 768
H, D = 12, 64
SCALE = D ** -0.5  # 0.125
NCORES = 8
KT = C // 128      # 6 k-tiles over the C contraction
QT = N // 512      # 2 q-tiles of 512
NKT = N // 128     # 8 k-tiles over sequence for attention

_CACHED = None


def _build():
    from contextlib import ExitStack

    from concourse import bacc
    import concourse.bass as bass
    import concourse.mybir as mybir
    from concourse.tile import TileContext
    from bass_rust import add_dep_helper

    f32 = mybir.dt.float32
    bf16 = mybir.dt.bfloat16
    Exp = mybir.ActivationFunctionType.Exp
    Alu = mybir.AluOpType

    nc = bacc.Bacc("TRN2", target_bir_lowering=False, debug=False)

    xT = nc.dram_tensor("xT", [C, N], bf16, kind="ExternalInput").ap()
    wqk = nc.dram_tensor("wqk", [H, 128, KT, 128], bf16, kind="ExternalInput").ap()
    wv = nc.dram_tensor("wv", [C, C], bf16, kind="ExternalInput").ap()
    wp = nc.dram_tensor("wp", [C, C], bf16, kind="ExternalInput").ap()
    bT = nc.dram_tensor("bT", [128, KT], f32, kind="ExternalInput").ap()
    yT = nc.dram_tensor("yT", [C, N], f32, kind="ExternalOutput").ap()
    la_dram = nc.dram_tensor("la_scratch", [H, N], f32).ap()
    ra_dram = nc.dram_tensor("ra_scratch", [H, N], f32).ap()

    with TileContext(nc) as tc, ExitStack() as ctx:
        singles = ctx.enter_context(tc.tile_pool(name="singles", bufs=1))
        put_pool = ctx.enter_context(tc.tile_pool(name="put", bufs=18))
        y_pool = ctx.enter_context(tc.tile_pool(name="y", bufs=3))
        rb_pool = ctx.enter_context(tc.tile_pool(name="rb", bufs=4))
        lst_pool = ctx.enter_context(tc.tile_pool(name="lst", bufs=4))
        la_pool = ctx.enter_context(tc.tile_pool(name="la", bufs=4))
        # PSUM: st (2 tiles x 2 banks) + o (2 x 1) + mm (2 x 1) = 8 banks
        mm_ps = ctx.enter_context(tc.tile_pool(name="mm_ps", bufs=2, space="PSUM"))
        o_ps = ctx.enter_context(tc.tile_pool(name="o_ps", bufs=2, space="PSUM"))
        st_ps = ctx.enter_context(tc.tile_pool(name="st_ps", bufs=2, space="PSUM"))

        # ---- persistent SBUF ----
        xT_sb = singles.tile([128, KT, N], bf16)          # 12 KB/part
        wqk_sb = singles.tile([128, H, KT, 128], bf16)    # 18 KB/part (m-major)
        wv_sb = singles.tile([128, KT, C], bf16)          # 9 KB/part
        wp_sb = singles.tile([128, KT, C], bf16)          # 9 KB/part
        bT_sb = singles.tile([128, KT], f32)
        qkT_sb = singles.tile([128, 2 * H, N], bf16)      # 24 KB/part
        v_sb = singles.tile([128, NKT, H, 65], bf16)      # 12.2 KB/part
        ouT_sb = singles.tile([128, KT, N], bf16)         # 12 KB/part
        yp_sb = singles.tile([128, H, 512], f32)          # 24 KB/part (proj partials)

        # ---- input DMAs, ordered by first use ----
        for k in range(KT):
            nc.sync.dma_start(out=xT_sb[:, k, :], in_=xT[k * 128:(k + 1) * 128, :])

        def wqk_dma(t):
            for m in (t, (H // 2) + t):
                nc.sync.dma_start(out=wqk_sb[:, m], in_=wqk[m])

        wqk_dma(0)
        for k in range(KT):  # wv chunk c0=0 (heads 0-7)
            nc.sync.dma_start(out=wv_sb[:, k, 0:512], in_=wv[k * 128:(k + 1) * 128, 0:512])
        wqk_dma(1)
        for k in range(KT):  # wv chunk c0=512 (heads 8-11)
            nc.sync.dma_start(out=wv_sb[:, k, 512:768], in_=wv[k * 128:(k + 1) * 128, 512:768])
        for t in range(2, KT):
            wqk_dma(t)
        for k in range(KT):
            nc.sync.dma_start(out=wp_sb[:, k, :], in_=wp[k * 128:(k + 1) * 128, :])
        nc.sync.dma_start(out=bT_sb, in_=bT[:, :])

        # ones column for the softmax-denominator trick
        nc.vector.memset(v_sb[:, :, :, 64:65], 1.0)

        def low_priority(thunk, bump=200000):
            save = tc.cur_priority
            tc.cur_priority = save + bump
            thunk()
            tc.cur_priority = save

        def qk_unit(t, u):
            """One (m, qt) unit of the qk-pass for head pair t (u in 0..3)."""
            m = t if u < 2 else (H // 2) + t
            qt = u % 2
            ps = mm_ps.tile([128, 512], f32, tag="mm", name=f"qk_{m}_{qt}")
            for k in range(KT):
                nc.tensor.matmul(
                    ps,
                    wqk_sb[:, m, k, :],
                    xT_sb[:, k, qt * 512:(qt + 1) * 512],
                    start=(k == 0),
                    stop=(k == KT - 1),
                )
            nc.vector.tensor_copy(qkT_sb[:, m, qt * 512:(qt + 1) * 512], ps)

        def v_chunk(j, c0, csz):
            """v[n-tile j, c0:c0+csz] = x @ w_v chunk (natural, n on partitions)."""
            ps = mm_ps.tile([128, 512], f32, tag="mm", name=f"v_{j}_{c0}")
            for k in range(KT):
                nc.tensor.matmul(
                    ps[:, 0:csz],
                    xT_sb[:, k, j * 128:(j + 1) * 128],
                    wv_sb[:, k, c0:c0 + csz],
                    start=(k == 0),
                    stop=(k == KT - 1),
                )
            nh = csz // 64
            nc.vector.tensor_copy(
                v_sb[:, j, c0 // 64:c0 // 64 + nh, 0:64],
                ps[:, 0:csz].rearrange("p (h c) -> p h c", c=64),
            )

        def drain_o(t, qt, tiles):
            """Copy O^T rows to ouT, extract l row, run the (1/l) chain for
            this q-tile half, and normalize the half in place."""
            he, ho = 2 * t, 2 * t + 1
            q0 = qt * 512
            la_writes = []
            for h, po in ((he, 0), (ho, 64)):
                nc.vector.tensor_copy(
                    ouT_sb[po:po + 64, t, q0:q0 + 512], tiles[h][0:64, :]
                )
                lst = lst_pool.tile([65, 512], f32, tag="lst", name=f"l_h{h}_q{qt}")
                nc.vector.tensor_copy(lst[64:65, :], tiles[h][64:65, :])
                la_writes.append(nc.sync.dma_start(
                    out=la_dram[h:h + 1, q0:q0 + 512], in_=lst[64:65, :]
                ))
            lv = la_dram[he:he + 2, q0:q0 + 512].rearrange("h (r c) -> h r c", c=128)
            rv = ra_dram[he:he + 2, q0:q0 + 512].rearrange("h (r c) -> h r c", c=128)
            la_t = la_pool.tile([8, 128], f32, tag="la", name=f"la_{t}_{qt}")
            ra_t = la_pool.tile([8, 128], f32, tag="ra", name=f"ra_{t}_{qt}")
            la_rd = nc.sync.dma_start(out=la_t, in_=lv)
            for w in la_writes:
                add_dep_helper(la_rd.ins, w.ins, reason="la dram write->read")
            nc.vector.reciprocal(ra_t, la_t)
            ra_wr = nc.sync.dma_start(out=rv, in_=ra_t)
            rb = rb_pool.tile([128, 512], f32, tag="rb", name=f"rb_{t}_{qt}")
            b1 = nc.sync.dma_start(
                out=rb[0:64, :], in_=ra_dram[he:he + 1, q0:q0 + 512].to_broadcast([64, 512])
            )
            b2 = nc.sync.dma_start(
                out=rb[64:128, :], in_=ra_dram[ho:ho + 1, q0:q0 + 512].to_broadcast([64, 512])
            )
            add_dep_helper(b1.ins, ra_wr.ins, reason="ra dram write->read")
            add_dep_helper(b2.ins, ra_wr.ins, reason="ra dram write->read")
            nc.vector.tensor_mul(
                ouT_sb[:, t, q0:q0 + 512], ouT_sb[:, t, q0:q0 + 512], rb
            )

        def attention_pair(t, extras_by_j=None):
            he, ho = 2 * t, 2 * t + 1
            mt_q, mt_k = t, (H // 2) + t
            pu_tiles = {}
            # sweep A: ST + exp (ACT-bound) + O for q-tile 0
            o_tiles = {
                h: o_ps.tile([65, 512], f32, tag="o", name=f"o_h{h}_q0")
                for h in (he, ho)
            }
            for j in range(NKT):
                sts = {
                    h: st_ps.tile([128, N], f32, tag="st", name=f"st_h{h}_j{j}")
                    for h in (he, ho)
                }
                # alternate row groups (he: partitions 0-63, ho: 64-127)
                for qt in range(QT):
                    for h, po in ((he, 0), (ho, 64)):
                        nc.tensor.matmul(
                            sts[h][:, qt * 512:(qt + 1) * 512],
                            qkT_sb[po:po + 64, mt_k, j * 128:(j + 1) * 128],
                            qkT_sb[po:po + 64, mt_q, qt * 512:(qt + 1) * 512],
                            start=True,
                            stop=True,
                        )
                for h in (he, ho):
                    pu = put_pool.tile([128, N], bf16, tag="pu", name=f"pu_h{h}_j{j}")
                    nc.scalar.activation(pu, sts[h], Exp, scale=SCALE)
                    pu_tiles[(h, j)] = pu
                if extras_by_j and j in extras_by_j:
                    for thunk in extras_by_j[j]:
                        thunk()
                for h in (he, ho):
                    nc.tensor.matmul(
                        o_tiles[h],
                        v_sb[:, j, h, :],
                        pu_tiles[(h, j)][:, 0:512],
                        start=(j == 0),
                        stop=(j == NKT - 1),
                    )
            drain_o(t, 0, o_tiles)
            # sweep B: O for q-tile 1 (re-reads retained PuT tiles)
            o_tiles2 = {
                h: o_ps.tile([65, 512], f32, tag="o", name=f"o_h{h}_q1")
                for h in (he, ho)
            }
            for j in range(NKT):
                for h in (he, ho):
                    nc.tensor.matmul(
                        o_tiles2[h],
                        v_sb[:, j, h, :],
                        pu_tiles[(h, j)][:, 512:1024],
                        start=(j == 0),
                        stop=(j == NKT - 1),
                    )
            drain_o(t, 1, o_tiles2)

        def proj_partial():
            """Accumulate proj k-tiles 0..4 into SBUF partials (fills PE gaps
            in pair 5's ACT-bound phase)."""
            for m in range(KT):
                for qt in range(QT):
                    ps = mm_ps.tile([128, 512], f32, tag="mm", name=f"yp_{m}_{qt}")
                    for k in range(KT - 1):
                        nc.tensor.matmul(
                            ps,
                            wp_sb[:, k, m * 128:(m + 1) * 128],
                            ouT_sb[:, k, qt * 512:(qt + 1) * 512],
                            start=(k == 0),
                            stop=(k == KT - 2),
                        )
                    nc.vector.tensor_copy(yp_sb[:, m * QT + qt, :], ps)

        def proj_final():
            for m in range(KT):
                for qt in range(QT):
                    pool = mm_ps if (m * QT + qt) % 2 == 0 else o_ps
                    tag = "mm" if (m * QT + qt) % 2 == 0 else "o"
                    ps = pool.tile([128, 512], f32, tag=tag, name=f"y_{m}_{qt}")
                    k = KT - 1
                    nc.tensor.matmul(
                        ps,
                        wp_sb[:, k, m * 128:(m + 1) * 128],
                        ouT_sb[:, k, qt * 512:(qt + 1) * 512],
                        start=True,
                        stop=True,
                    )
                    yt = y_pool.tile([128, 512], f32, tag="y")
                    # yt = (ps + bias) + yp
                    nc.vector.scalar_tensor_tensor(
                        out=yt,
                        in0=ps,
                        scalar=bT_sb[:, m:m + 1],
                        in1=yp_sb[:, m * QT + qt, :],
                        op0=Alu.add,
                        op1=Alu.add,
                    )
                    nc.sync.dma_start(
                        out=yT[m * 128:(m + 1) * 128, qt * 512:(qt + 1) * 512], in_=yt
                    )

        # ---- schedule: qk(t+1) and v chunks are emitted inside pair t's
        # j-loop so the PE fills ACT-bound gaps and qkT(t+1) is ready at the
        # pair boundary ----
        def extras(t):
            e = {j: [] for j in range(NKT)}
            if t == 0:
                for j in range(NKT):
                    e[j].append(lambda j=j: v_chunk(j, 0, 512))
            if t == 2:
                for j in range(NKT):
                    e[j].append(lambda j=j: v_chunk(j, 512, 256))
            if t < KT - 1:
                for u, j in enumerate((1, 3, 5, 7)):
                    e[j].append(lambda t=t, u=u: qk_unit(t + 1, u))
            return e

        for u in range(4):
            qk_unit(0, u)
        for t in range(KT):
            attention_pair(t, extras(t))
        proj_partial()
        proj_final()

    nc.compile()
    return nc


def _get_nc():
    global _CACHED
    if _CACHED is None:
        _CACHED = _build()
    return _CACHED


def kernel(x, w_qkv, w_proj, b_proj):
    import ml_dtypes
    from concourse.bass_utils import run_bass_kernel_spmd

    x = np.asarray(x, dtype=np.float32)
    w_qkv = np.asarray(w_qkv, dtype=np.float32)
    w_proj = np.asarray(w_proj, dtype=np.float32)
    b_proj = np.asarray(b_proj, dtype=np.float32)

    nc = _get_nc()

    wqk_t = np.ascontiguousarray(
        w_qkv[:, : 2 * C].astype(ml_dtypes.bfloat16)
        .reshape(KT, 128, H, 128).transpose(2, 1, 0, 3)
    )
    wv = np.ascontiguousarray(w_qkv[:, 2 * C:].astype(ml_dtypes.bfloat16))
    wp = np.ascontiguousarray(w_proj.astype(ml_dtypes.bfloat16))
    bT = np.ascontiguousarray(b_proj.reshape(KT, 128).T)

    in_maps = []
    for b in range(B):
        in_maps.append(
            {
                "xT": np.ascontiguousarray(x[b].T.astype(ml_dtypes.bfloat16)),
                "wqk": wqk_t,
                "wv": wv,
                "wp": wp,
                "bT": bT,
            }
        )

    res = run_bass_kernel_spmd(nc, in_maps, list(range(NCORES)))
    out = np.empty((B, N, C), dtype=np.float32)
    for b in range(B):
        out[b] = res.results[b]["yT"].T
    return out


# revision 25
# speedup vs baseline: 1.0026x; 1.0026x over previous
"""Trainium2 Bass kernel for multi-head attention (B=8, N=1024, C=768, H=12, D=64).

Sharding: pure data parallelism — one batch element per NeuronCore (8 cores).
Each core computes qkv projection, softmax attention and output projection for
its [1024, 768] slice with full (replicated) weights. No collectives.

Dataflow (all "transposed" so no on-device transposes are needed):
  - host supplies xT = x[b].T (bf16) and w_qk pre-tiled [KT, 2H, 128, 128]
  - qk-pass:  qkT = w_qk.T @ x.T -> [1536, 1024]; head-pair t's tiles are
              computed during attention of pair t-1 (PE gap filling)
  - v-pass:   v = x @ w_v natural [1024, 768], computed inside pair 0/2 loops
  - ST pass:  ST[k,q] = (K Q^T) per head
  - exp:      PuT = exp(SCALE * ST) on ScalarE (no max subtraction: |S|<~7)
  - O pass:   O^T[d,q]: lhsT=[v | ones] so PSUM row 64 accumulates the softmax
              denominator l; two q-tile sweeps to keep PSUM pressure low
  - normalize: per head pair and q-tile, O^T *= (1/l)
  - proj:     yT = w_proj.T @ O^T + b; k-tiles 0..4 pre-accumulated and spilled
              so only the k=5 matmuls wait for the last attention pair
  - host transposes yT back.
"""

import sys

sys.path.insert(0, "/opt/trn_rl_repo")

import numpy as np

B, N, C = 8, 1024, 768
H, D = 12, 64
SCALE = D ** -0.5  # 0.125
NCORES = 8
KT = C // 128      # 6 k-tiles over the C contraction
QT = N // 512      # 2 q-tiles of 512
NKT = N // 128     # 8 k-tiles over sequence for attention

_CACHED = None


def _build():
    from contextlib import ExitStack

    from concourse import bacc
    import concourse.bass as bass
    import concourse.mybir as mybir
    from concourse.tile import TileContext
    from bass_rust import add_dep_helper

    f32 = mybir.dt.float32
    bf16 = mybir.dt.bfloat16
    Exp = mybir.ActivationFunctionType.Exp
    Alu = mybir.AluOpType

    nc = bacc.Bacc("TRN2", target_bir_lowering=False, debug=False)

    xT = nc.dram_tensor("xT", [C, N], bf16, kind="ExternalInput").ap()
    wqk = nc.dram_tensor("wqk", [H, 128, KT, 128], bf16, kind="ExternalInput").ap()
    wv = nc.dram_tensor("wv", [C, C], bf16, kind="ExternalInput").ap()
    wp = nc.dram_tensor("wp", [C, C], bf16, kind="ExternalInput").ap()
    bT = nc.dram_tensor("bT", [128, KT], f32, kind="ExternalInput").ap()
    yT = nc.dram_tensor("yT", [C, N], f32, kind="ExternalOutput").ap()
    la_dram = nc.dram_tensor("la_scratch", [H, N], f32).ap()
    ra_dram = nc.dram_tensor("ra_scratch", [H, N], f32).ap()

    with TileContext(nc) as tc, ExitStack() as ctx:
        singles = ctx.enter_context(tc.tile_pool(name="singles", bufs=1))
        put_pool = ctx.enter_context(tc.tile_pool(name="put", bufs=18))
        y_pool = ctx.enter_context(tc.tile_pool(name="y", bufs=3))
        rb_pool = ctx.enter_context(tc.tile_pool(name="rb", bufs=4))
        lst_pool = ctx.enter_context(tc.tile_pool(name="lst", bufs=4))
        la_pool = ctx.enter_context(tc.tile_pool(name="la", bufs=4))
        # PSUM: st (2 tiles x 2 banks) + o (2 x 1) + mm (2 x 1) = 8 banks
        mm_ps = ctx.enter_context(tc.tile_pool(name="mm_ps", bufs=2, space="PSUM"))
        o_ps = ctx.enter_context(tc.tile_pool(name="o_ps", bufs=2, space="PSUM"))
        st_ps = ctx.enter_context(tc.tile_pool(name="st_ps", bufs=2, space="PSUM"))

        # ---- persistent SBUF ----
        xT_sb = singles.tile([128, KT, N], bf16)          # 12 KB/part
        wqk_sb = singles.tile([128, H, KT, 128], bf16)    # 18 KB/part (m-major)
        wv_sb = singles.tile([128, KT, C], bf16)          # 9 KB/part
        wp_sb = singles.tile([128, KT, C], bf16)          # 9 KB/part
        bT_sb = singles.tile([128, KT], f32)
        qkT_sb = singles.tile([128, 2 * H, N], bf16)      # 24 KB/part
        v_sb = singles.tile([128, NKT, H, 65], bf16)      # 12.2 KB/part
        ouT_sb = singles.tile([128, KT, N], bf16)         # 12 KB/part
        yp_sb = singles.tile([128, H, 512], f32)          # 24 KB/part (proj partials)

        # ---- input DMAs, ordered by first use ----
        for k in range(KT):
            nc.sync.dma_start(out=xT_sb[:, k, :], in_=xT[k * 128:(k + 1) * 128, :])

        def wqk_dma(t):
            for m in (t, (H // 2) + t):
                nc.sync.dma_start(out=wqk_sb[:, m], in_=wqk[m])

        wqk_dma(0)
        for k in range(KT):  # wv chunk c0=0 (heads 0-7)
            nc.sync.dma_start(out=wv_sb[:, k, 0:512], in_=wv[k * 128:(k + 1) * 128, 0:512])
        wqk_dma(1)
        for k in range(KT):  # wv chunk c0=512 (heads 8-11)
            nc.sync.dma_start(out=wv_sb[:, k, 512:768], in_=wv[k * 128:(k + 1) * 128, 512:768])
        for t in range(2, KT):
            wqk_dma(t)
        for k in range(KT):
            nc.sync.dma_start(out=wp_sb[:, k, :], in_=wp[k * 128:(k + 1) * 128, :])
        nc.sync.dma_start(out=bT_sb, in_=bT[:, :])

        # ones column for the softmax-denominator trick
        nc.vector.memset(v_sb[:, :, :, 64:65], 1.0)

        def low_priority(thunk, bump=200000):
            save = tc.cur_priority
            tc.cur_priority = save + bump
            thunk()
            tc.cur_priority = save

        def qk_unit(t, u):
            """One (m, qt) unit of the qk-pass for head pair t (u in 0..3)."""
            m = t if u < 2 else (H // 2) + t
            qt = u % 2
            ps = mm_ps.tile([128, 512], f32, tag="mm", name=f"qk_{m}_{qt}")
            for k in range(KT):
                nc.tensor.matmul(
                    ps,
                    wqk_sb[:, m, k, :],
                    xT_sb[:, k, qt * 512:(qt + 1) * 512],
                    start=(k == 0),
                    stop=(k == KT - 1),
                )
            nc.vector.tensor_copy(qkT_sb[:, m, qt * 512:(qt + 1) * 512], ps)

        def v_chunk(j, c0, csz):
            """v[n-tile j, c0:c0+csz] = x @ w_v chunk (natural, n on partitions)."""
            ps = mm_ps.tile([128, 512], f32, tag="mm", name=f"v_{j}_{c0}")
            for k in range(KT):
                nc.tensor.matmul(
                    ps[:, 0:csz],
                    xT_sb[:, k, j * 128:(j + 1) * 128],
                    wv_sb[:, k, c0:c0 + csz],
                    start=(k == 0),
                    stop=(k == KT - 1),
                )
            nh = csz // 64
            nc.vector.tensor_copy(
                v_sb[:, j, c0 // 64:c0 // 64 + nh, 0:64],
                ps[:, 0:csz].rearrange("p (h c) -> p h c", c=64),
            )

        def drain_o(t, qt, tiles):
            """Copy O^T rows to ouT, extract l row, run the (1/l) chain for
            this q-tile half, and normalize the half in place."""
            he, ho = 2 * t, 2 * t + 1
            q0 = qt * 512
            la_writes = []
            for h, po in ((he, 0), (ho, 64)):
                nc.vector.tensor_copy(
                    ouT_sb[po:po + 64, t, q0:q0 + 512], tiles[h][0:64, :]
                )
                lst = lst_pool.tile([65, 512], f32, tag="lst", name=f"l_h{h}_q{qt}")
                nc.vector.tensor_copy(lst[64:65, :], tiles[h][64:65, :])
                la_writes.append(nc.sync.dma_start(
                    out=la_dram[h:h + 1, q0:q0 + 512], in_=lst[64:65, :]
                ))
            lv = la_dram[he:he + 2, q0:q0 + 512].rearrange("h (r c) -> h r c", c=128)
            rv = ra_dram[he:he + 2, q0:q0 + 512].rearrange("h (r c) -> h r c", c=128)
            la_t = la_pool.tile([8, 128], f32, tag="la", name=f"la_{t}_{qt}")
            ra_t = la_pool.tile([8, 128], f32, tag="ra", name=f"ra_{t}_{qt}")
            la_rd = nc.sync.dma_start(out=la_t, in_=lv)
            for w in la_writes:
                add_dep_helper(la_rd.ins, w.ins, reason="la dram write->read")
            nc.vector.reciprocal(ra_t, la_t)
            ra_wr = nc.sync.dma_start(out=rv, in_=ra_t)
            rb = rb_pool.tile([128, 512], f32, tag="rb", name=f"rb_{t}_{qt}")
            b1 = nc.sync.dma_start(
                out=rb[0:64, :], in_=ra_dram[he:he + 1, q0:q0 + 512].to_broadcast([64, 512])
            )
            b2 = nc.sync.dma_start(
                out=rb[64:128, :], in_=ra_dram[ho:ho + 1, q0:q0 + 512].to_broadcast([64, 512])
            )
            add_dep_helper(b1.ins, ra_wr.ins, reason="ra dram write->read")
            add_dep_helper(b2.ins, ra_wr.ins, reason="ra dram write->read")
            nc.vector.tensor_mul(
                ouT_sb[:, t, q0:q0 + 512], ouT_sb[:, t, q0:q0 + 512], rb
            )

        def st_exp_step(t, j):
            """Emit the ST matmuls + exps for (pair t, k-tile j); returns pu tiles."""
            he, ho = 2 * t, 2 * t + 1
            mt_q, mt_k = t, (H // 2) + t
            sts = {
                h: st_ps.tile([128, N], f32, tag="st", name=f"st_h{h}_j{j}")
                for h in (he, ho)
            }
            # alternate row groups (he: partitions 0-63, ho: 64-127)
            for qt in range(QT):
                for h, po in ((he, 0), (ho, 64)):
                    nc.tensor.matmul(
                        sts[h][:, qt * 512:(qt + 1) * 512],
                        qkT_sb[po:po + 64, mt_k, j * 128:(j + 1) * 128],
                        qkT_sb[po:po + 64, mt_q, qt * 512:(qt + 1) * 512],
                        start=True,
                        stop=True,
                    )
            out = {}
            for h in (he, ho):
                pu = put_pool.tile([128, N], bf16, tag="pu", name=f"pu_h{h}_j{j}")
                nc.scalar.activation(pu, sts[h], Exp, scale=SCALE)
                out[h] = pu
            return out

        def attention_pair(t, extras_by_j=None, pulled=None, pull_next=False):
            he, ho = 2 * t, 2 * t + 1
            mt_q, mt_k = t, (H // 2) + t
            pu_tiles = {}
            # sweep A: ST + exp (ACT-bound) + O for q-tile 0
            o_tiles = {
                h: o_ps.tile([65, 512], f32, tag="o", name=f"o_h{h}_q0")
                for h in (he, ho)
            }
            for j in range(NKT):
                if pulled is not None and j == 0:
                    pus = pulled
                else:
                    pus = st_exp_step(t, j)
                for h in (he, ho):
                    pu_tiles[(h, j)] = pus[h]
                if extras_by_j and j in extras_by_j:
                    for thunk in extras_by_j[j]:
                        thunk()
                for h in (he, ho):
                    nc.tensor.matmul(
                        o_tiles[h],
                        v_sb[:, j, h, :],
                        pu_tiles[(h, j)][:, 0:512],
                        start=(j == 0),
                        stop=(j == NKT - 1),
                    )
            drain_o(t, 0, o_tiles)
            pulled_next = st_exp_step(t + 1, 0) if pull_next else None
            # sweep B: O for q-tile 1 (re-reads retained PuT tiles)
            o_tiles2 = {
                h: o_ps.tile([65, 512], f32, tag="o", name=f"o_h{h}_q1")
                for h in (he, ho)
            }
            for j in range(NKT):
                for h in (he, ho):
                    nc.tensor.matmul(
                        o_tiles2[h],
                        v_sb[:, j, h, :],
                        pu_tiles[(h, j)][:, 512:1024],
                        start=(j == 0),
                        stop=(j == NKT - 1),
                    )
            drain_o(t, 1, o_tiles2)
            return pulled_next

        def proj_partial():
            """Accumulate proj k-tiles 0..4 into SBUF partials (fills PE gaps
            in pair 5's ACT-bound phase)."""
            for m in range(KT):
                for qt in range(QT):
                    ps = mm_ps.tile([128, 512], f32, tag="mm", name=f"yp_{m}_{qt}")
                    for k in range(KT - 1):
                        nc.tensor.matmul(
                            ps,
                            wp_sb[:, k, m * 128:(m + 1) * 128],
                            ouT_sb[:, k, qt * 512:(qt + 1) * 512],
                            start=(k == 0),
                            stop=(k == KT - 2),
                        )
                    nc.vector.tensor_copy(yp_sb[:, m * QT + qt, :], ps)

        def proj_final():
            for m in range(KT):
                for qt in range(QT):
                    ps = mm_ps.tile([128, 512], f32, tag="mm", name=f"y_{m}_{qt}")
                    k = KT - 1
                    nc.tensor.matmul(
                        ps,
                        wp_sb[:, k, m * 128:(m + 1) * 128],
                        ouT_sb[:, k, qt * 512:(qt + 1) * 512],
                        start=True,
                        stop=True,
                    )
                    yt = y_pool.tile([128, 512], f32, tag="y")
                    # yt = (ps + bias) + yp
                    nc.vector.scalar_tensor_tensor(
                        out=yt,
                        in0=ps,
                        scalar=bT_sb[:, m:m + 1],
                        in1=yp_sb[:, m * QT + qt, :],
                        op0=Alu.add,
                        op1=Alu.add,
                    )
                    nc.sync.dma_start(
                        out=yT[m * 128:(m + 1) * 128, qt * 512:(qt + 1) * 512], in_=yt
                    )

        # ---- schedule: qk(t+1) and v chunks are emitted inside pair t's
        # j-loop so the PE fills ACT-bound gaps and qkT(t+1) is ready at the
        # pair boundary ----
        def extras(t):
            e = {j: [] for j in range(NKT)}
            if t == 0:
                for j in range(NKT):
                    e[j].append(lambda j=j: v_chunk(j, 0, 512))
            if t == 2:
                for j in range(NKT):
                    e[j].append(lambda j=j: v_chunk(j, 512, 256))
            if t < KT - 1:
                for u, j in enumerate((1, 3, 5, 7)):
                    e[j].append(lambda t=t, u=u: qk_unit(t + 1, u))
            return e

        for u in range(4):
            qk_unit(0, u)
        pulled = None
        for t in range(KT):
            pulled = attention_pair(t, extras(t), pulled, pull_next=(t < KT - 1))
        proj_partial()
        proj_final()

    nc.compile()
    return nc


def _get_nc():
    global _CACHED
    if _CACHED is None:
        _CACHED = _build()
    return _CACHED


def kernel(x, w_qkv, w_proj, b_proj):
    import ml_dtypes
    from concourse.bass_utils import run_bass_kernel_spmd

    x = np.asarray(x, dtype=np.float32)
    w_qkv = np.asarray(w_qkv, dtype=np.float32)
    w_proj = np.asarray(w_proj, dtype=np.float32)
    b_proj = np.asarray(b_proj, dtype=np.float32)

    nc = _get_nc()

    wqk_t = np.ascontiguousarray(
        w_qkv[:, : 2 * C].astype(ml_dtypes.bfloat16)
        .reshape(KT, 128, H, 128).transpose(2, 1, 0, 3)
    )
    wv = np.ascontiguousarray(w_qkv[:, 2 * C:].astype(ml_dtypes.bfloat16))
    wp = np.ascontiguousarray(w_proj.astype(ml_dtypes.bfloat16))
    bT = np.ascontiguousarray(b_proj.reshape(KT, 128).T)

    in_maps = []
    for b in range(B):
        in_maps.append(
            {
                "xT": np.ascontiguousarray(x[b].T.astype(ml_dtypes.bfloat16)),
                "wqk": wqk_t,
                "wv": wv,
                "wp": wp,
                "bT": bT,
            }
        )

    res = run_bass_kernel_spmd(nc, in_maps, list(range(NCORES)))
    out = np.empty((B, N, C), dtype=np.float32)
    for b in range(B):
        out[b] = res.results[b]["yT"].T
    return out


# revision 26
# speedup vs baseline: 1.0189x; 1.0162x over previous
"""Trainium2 Bass kernel for multi-head attention (B=8, N=1024, C=768, H=12, D=64).

Sharding: pure data parallelism — one batch element per NeuronCore (8 cores).
Each core computes qkv projection, softmax attention and output projection for
its [1024, 768] slice with full (replicated) weights. No collectives.

Dataflow (all "transposed" so no on-device transposes are needed):
  - host supplies xT = x[b].T (bf16) and w_qk pre-tiled [KT, 2H, 128, 128]
  - qk-pass:  qkT = w_qk.T @ x.T -> [1536, 1024]; head-pair t's tiles are
              computed during attention of pair t-1 (PE gap filling)
  - v-pass:   v = x @ w_v natural [1024, 768], computed inside pair 0/2 loops
  - ST pass:  ST[k,q] = (K Q^T) per head
  - exp:      PuT = exp(SCALE * ST) on ScalarE (no max subtraction: |S|<~7)
  - O pass:   O^T[d,q]: lhsT=[v | ones] so PSUM row 64 accumulates the softmax
              denominator l; two q-tile sweeps to keep PSUM pressure low
  - normalize: per head pair and q-tile, O^T *= (1/l)
  - proj:     yT = w_proj.T @ O^T + b; k-tiles 0..4 pre-accumulated and spilled
              so only the k=5 matmuls wait for the last attention pair
  - host transposes yT back.
"""

import sys

sys.path.insert(0, "/opt/trn_rl_repo")

import numpy as np

B, N, C = 8, 1024, 768
H, D = 12, 64
SCALE = D ** -0.5  # 0.125
NCORES = 8
KT = C // 128      # 6 k-tiles over the C contraction
QT = N // 512      # 2 q-tiles of 512
NKT = N // 128     # 8 k-tiles over sequence for attention

_CACHED = None


def _build():
    from contextlib import ExitStack

    from concourse import bacc
    import concourse.bass as bass
    import concourse.mybir as mybir
    from concourse.tile import TileContext
    from bass_rust import add_dep_helper

    f32 = mybir.dt.float32
    bf16 = mybir.dt.bfloat16
    Exp = mybir.ActivationFunctionType.Exp
    Alu = mybir.AluOpType

    nc = bacc.Bacc("TRN2", target_bir_lowering=False, debug=False)

    xT = nc.dram_tensor("xT", [C, N], bf16, kind="ExternalInput").ap()
    wqk = nc.dram_tensor("wqk", [H, 128, KT, 128], bf16, kind="ExternalInput").ap()
    wv = nc.dram_tensor("wv", [C, C], bf16, kind="ExternalInput").ap()
    wp = nc.dram_tensor("wp", [C, C], bf16, kind="ExternalInput").ap()
    bT = nc.dram_tensor("bT", [128, KT], f32, kind="ExternalInput").ap()
    yT = nc.dram_tensor("yT", [C, N], f32, kind="ExternalOutput").ap()
    la_dram = nc.dram_tensor("la_scratch", [H, N], f32).ap()
    ra_dram = nc.dram_tensor("ra_scratch", [H, N], f32).ap()

    with TileContext(nc) as tc, ExitStack() as ctx:
        singles = ctx.enter_context(tc.tile_pool(name="singles", bufs=1))
        put_pool = ctx.enter_context(tc.tile_pool(name="put", bufs=18))
        y_pool = ctx.enter_context(tc.tile_pool(name="y", bufs=3))
        rb_pool = ctx.enter_context(tc.tile_pool(name="rb", bufs=4))
        lst_pool = ctx.enter_context(tc.tile_pool(name="lst", bufs=4))
        la_pool = ctx.enter_context(tc.tile_pool(name="la", bufs=4))
        # PSUM: st (2 tiles x 2 banks) + o (2 x 1) + mm (2 x 1) = 8 banks
        mm_ps = ctx.enter_context(tc.tile_pool(name="mm_ps", bufs=2, space="PSUM"))
        o_ps = ctx.enter_context(tc.tile_pool(name="o_ps", bufs=2, space="PSUM"))
        st_ps = ctx.enter_context(tc.tile_pool(name="st_ps", bufs=2, space="PSUM"))

        # ---- persistent SBUF ----
        xT_sb = singles.tile([128, KT, N], bf16)          # 12 KB/part
        wqk_sb = singles.tile([128, H, KT, 128], bf16)    # 18 KB/part (m-major)
        wv_sb = singles.tile([128, KT, C], bf16)          # 9 KB/part
        wp_sb = singles.tile([128, KT, C], bf16)          # 9 KB/part
        bT_sb = singles.tile([128, KT], f32)
        qkT_sb = singles.tile([128, 2 * H, N], bf16)      # 24 KB/part
        v_sb = singles.tile([128, NKT, H, 65], bf16)      # 12.2 KB/part
        ouT_sb = singles.tile([128, KT, N], bf16)         # 12 KB/part
        yp_sb = singles.tile([128, H, 512], f32)          # 24 KB/part (proj partials)

        # ---- input DMAs, ordered by first use ----
        for k in range(KT):
            nc.sync.dma_start(out=xT_sb[:, k, :], in_=xT[k * 128:(k + 1) * 128, :])

        def wqk_dma(t):
            for m in (t, (H // 2) + t):
                nc.sync.dma_start(out=wqk_sb[:, m], in_=wqk[m])

        wqk_dma(0)
        for k in range(KT):  # wv chunk c0=0 (heads 0-7)
            nc.sync.dma_start(out=wv_sb[:, k, 0:512], in_=wv[k * 128:(k + 1) * 128, 0:512])
        wqk_dma(1)
        for k in range(KT):  # wv chunk c0=512 (heads 8-11)
            nc.sync.dma_start(out=wv_sb[:, k, 512:768], in_=wv[k * 128:(k + 1) * 128, 512:768])
        for t in range(2, KT):
            wqk_dma(t)
        for k in range(KT):
            nc.sync.dma_start(out=wp_sb[:, k, :], in_=wp[k * 128:(k + 1) * 128, :])
        nc.sync.dma_start(out=bT_sb, in_=bT[:, :])

        # ones column for the softmax-denominator trick
        nc.vector.memset(v_sb[:, :, :, 64:65], 1.0)

        def low_priority(thunk, bump=200000):
            save = tc.cur_priority
            tc.cur_priority = save + bump
            thunk()
            tc.cur_priority = save

        def qk_unit(t, u):
            """One (m, qt) unit of the qk-pass for head pair t (u in 0..3)."""
            m = t if u < 2 else (H // 2) + t
            qt = u % 2
            ps = mm_ps.tile([128, 512], f32, tag="mm", name=f"qk_{m}_{qt}")
            for k in range(KT):
                nc.tensor.matmul(
                    ps,
                    wqk_sb[:, m, k, :],
                    xT_sb[:, k, qt * 512:(qt + 1) * 512],
                    start=(k == 0),
                    stop=(k == KT - 1),
                )
            nc.vector.tensor_copy(qkT_sb[:, m, qt * 512:(qt + 1) * 512], ps)

        def v_chunk(j, c0, csz):
            """v[n-tile j, c0:c0+csz] = x @ w_v chunk (natural, n on partitions)."""
            ps = mm_ps.tile([128, 512], f32, tag="mm", name=f"v_{j}_{c0}")
            for k in range(KT):
                nc.tensor.matmul(
                    ps[:, 0:csz],
                    xT_sb[:, k, j * 128:(j + 1) * 128],
                    wv_sb[:, k, c0:c0 + csz],
                    start=(k == 0),
                    stop=(k == KT - 1),
                )
            nh = csz // 64
            nc.vector.tensor_copy(
                v_sb[:, j, c0 // 64:c0 // 64 + nh, 0:64],
                ps[:, 0:csz].rearrange("p (h c) -> p h c", c=64),
            )

        def drain_o(t, qt, tiles):
            """Copy O^T rows to ouT, extract l row, run the (1/l) chain for
            this q-tile half, and normalize the half in place."""
            he, ho = 2 * t, 2 * t + 1
            q0 = qt * 512
            la_writes = []
            for h, po in ((he, 0), (ho, 64)):
                nc.vector.tensor_copy(
                    ouT_sb[po:po + 64, t, q0:q0 + 512], tiles[h][0:64, :]
                )
                lst = lst_pool.tile([65, 512], f32, tag="lst", name=f"l_h{h}_q{qt}")
                nc.vector.tensor_copy(lst[64:65, :], tiles[h][64:65, :])
                la_writes.append(nc.sync.dma_start(
                    out=la_dram[h:h + 1, q0:q0 + 512], in_=lst[64:65, :]
                ))
            lv = la_dram[he:he + 2, q0:q0 + 512].rearrange("h (r c) -> h r c", c=128)
            rv = ra_dram[he:he + 2, q0:q0 + 512].rearrange("h (r c) -> h r c", c=128)
            la_t = la_pool.tile([8, 128], f32, tag="la", name=f"la_{t}_{qt}")
            ra_t = la_pool.tile([8, 128], f32, tag="ra", name=f"ra_{t}_{qt}")
            la_rd = nc.sync.dma_start(out=la_t, in_=lv)
            for w in la_writes:
                add_dep_helper(la_rd.ins, w.ins, reason="la dram write->read")
            nc.vector.reciprocal(ra_t, la_t)
            ra_wr = nc.sync.dma_start(out=rv, in_=ra_t)
            rb = rb_pool.tile([128, 512], f32, tag="rb", name=f"rb_{t}_{qt}")
            b1 = nc.sync.dma_start(
                out=rb[0:64, :], in_=ra_dram[he:he + 1, q0:q0 + 512].to_broadcast([64, 512])
            )
            b2 = nc.sync.dma_start(
                out=rb[64:128, :], in_=ra_dram[ho:ho + 1, q0:q0 + 512].to_broadcast([64, 512])
            )
            add_dep_helper(b1.ins, ra_wr.ins, reason="ra dram write->read")
            add_dep_helper(b2.ins, ra_wr.ins, reason="ra dram write->read")
            nc.vector.tensor_mul(
                ouT_sb[:, t, q0:q0 + 512], ouT_sb[:, t, q0:q0 + 512], rb
            )

        def attention_pair(t, extras_by_j=None):
            he, ho = 2 * t, 2 * t + 1
            mt_q, mt_k = t, (H // 2) + t
            pu_tiles = {}
            # sweep A: ST + exp (ACT-bound) + O for q-tile 0
            o_tiles = {
                h: o_ps.tile([65, 512], f32, tag="o", name=f"o_h{h}_q0")
                for h in (he, ho)
            }
            for j in range(NKT):
                sts = {
                    h: st_ps.tile([128, N], f32, tag="st", name=f"st_h{h}_j{j}")
                    for h in (he, ho)
                }
                # alternate row groups (he: partitions 0-63, ho: 64-127)
                for qt in range(QT):
                    for h, po in ((he, 0), (ho, 64)):
                        nc.tensor.matmul(
                            sts[h][:, qt * 512:(qt + 1) * 512],
                            qkT_sb[po:po + 64, mt_k, j * 128:(j + 1) * 128],
                            qkT_sb[po:po + 64, mt_q, qt * 512:(qt + 1) * 512],
                            start=True,
                            stop=True,
                        )
                for h in (he, ho):
                    pu = put_pool.tile([128, N], bf16, tag="pu", name=f"pu_h{h}_j{j}")
                    nc.scalar.activation(pu, sts[h], Exp, scale=SCALE)
                    pu_tiles[(h, j)] = pu
                if extras_by_j and j in extras_by_j:
                    for thunk in extras_by_j[j]:
                        thunk()
                for h in (he, ho):
                    nc.tensor.matmul(
                        o_tiles[h],
                        v_sb[:, j, h, :],
                        pu_tiles[(h, j)][:, 0:512],
                        start=(j == 0),
                        stop=(j == NKT - 1),
                    )
            drain_o(t, 0, o_tiles)
            # sweep B: O for q-tile 1 (re-reads retained PuT tiles)
            o_tiles2 = {
                h: o_ps.tile([65, 512], f32, tag="o", name=f"o_h{h}_q1")
                for h in (he, ho)
            }
            for j in range(NKT):
                for h in (he, ho):
                    nc.tensor.matmul(
                        o_tiles2[h],
                        v_sb[:, j, h, :],
                        pu_tiles[(h, j)][:, 512:1024],
                        start=(j == 0),
                        stop=(j == NKT - 1),
                    )
            drain_o(t, 1, o_tiles2)

        def proj_partial():
            """Accumulate proj k-tiles 0..4 into SBUF partials (fills PE gaps
            in pair 5's ACT-bound phase)."""
            for m in range(KT):
                for qt in range(QT):
                    ps = mm_ps.tile([128, 512], f32, tag="mm", name=f"yp_{m}_{qt}")
                    for k in range(KT - 1):
                        nc.tensor.matmul(
                            ps,
                            wp_sb[:, k, m * 128:(m + 1) * 128],
                            ouT_sb[:, k, qt * 512:(qt + 1) * 512],
                            start=(k == 0),
                            stop=(k == KT - 2),
                        )
                    nc.vector.tensor_copy(yp_sb[:, m * QT + qt, :], ps)

        def proj_final():
            for m in range(KT):
                for qt in range(QT):
                    ps = mm_ps.tile([128, 512], f32, tag="mm", name=f"y_{m}_{qt}")
                    k = KT - 1
                    nc.tensor.matmul(
                        ps,
                        wp_sb[:, k, m * 128:(m + 1) * 128],
                        ouT_sb[:, k, qt * 512:(qt + 1) * 512],
                        start=True,
                        stop=True,
                    )
                    yt = y_pool.tile([128, 512], f32, tag="y")
                    # yt = (ps + bias) + yp
                    nc.vector.scalar_tensor_tensor(
                        out=yt,
                        in0=ps,
                        scalar=bT_sb[:, m:m + 1],
                        in1=yp_sb[:, m * QT + qt, :],
                        op0=Alu.add,
                        op1=Alu.add,
                    )
                    nc.sync.dma_start(
                        out=yT[m * 128:(m + 1) * 128, qt * 512:(qt + 1) * 512], in_=yt
                    )

        # ---- schedule: qk(t+1) and v chunks are emitted inside pair t's
        # j-loop so the PE fills ACT-bound gaps and qkT(t+1) is ready at the
        # pair boundary ----
        def extras(t):
            e = {j: [] for j in range(NKT)}
            if t == 0:
                for j in range(NKT):
                    e[j].append(lambda j=j: v_chunk(j, 0, 512))
            if t == 2:
                for j in range(NKT):
                    e[j].append(lambda j=j: v_chunk(j, 512, 256))
            if t < KT - 1:
                for u, j in enumerate((1, 3, 5, 7)):
                    e[j].append(lambda t=t, u=u: qk_unit(t + 1, u))
            return e

        for u in range(4):
            qk_unit(0, u)
        for t in range(KT):
            attention_pair(t, extras(t))
        proj_partial()
        proj_final()

    nc.compile()
    return nc


def _get_nc():
    global _CACHED
    if _CACHED is None:
        _CACHED = _build()
    return _CACHED


def kernel(x, w_qkv, w_proj, b_proj):
    import ml_dtypes
    from concourse.bass_utils import run_bass_kernel_spmd

    x = np.asarray(x, dtype=np.float32)
    w_qkv = np.asarray(w_qkv, dtype=np.float32)
    w_proj = np.asarray(w_proj, dtype=np.float32)
    b_proj = np.asarray(b_proj, dtype=np.float32)

    nc = _get_nc()

    wqk_t = np.ascontiguousarray(
        w_qkv[:, : 2 * C].astype(ml_dtypes.bfloat16)
        .reshape(KT, 128, H, 128).transpose(2, 1, 0, 3)
    )
    wv = np.ascontiguousarray(w_qkv[:, 2 * C:].astype(ml_dtypes.bfloat16))
    wp = np.ascontiguousarray(w_proj.astype(ml_dtypes.bfloat16))
    bT = np.ascontiguousarray(b_proj.reshape(KT, 128).T)

    in_maps = []
    for b in range(B):
        in_maps.append(
            {
                "xT": np.ascontiguousarray(x[b].T.astype(ml_dtypes.bfloat16)),
                "wqk": wqk_t,
                "wv": wv,
                "wp": wp,
                "bT": bT,
            }
        )

    res = run_bass_kernel_spmd(nc, in_maps, list(range(NCORES)))
    out = np.empty((B, N, C), dtype=np.float32)
    for b in range(B):
        out[b] = res.results[b]["yT"].T
    return out
